# revision 1
# baseline (speedup 1.0000x reference)
"""Trainium2 Bass kernel for nn_MultiHeadMALAAttention.

Sharding: 8 cores; core c handles batch b = c//2, token half h = c%2
(tokens [h*4096, (h+1)*4096) of N=8192).  Stats (kmean, vmean, kv_state)
need full-N reductions -> pairwise AllReduce between the two cores of a
batch, replica groups [[0,1],[2,3],[4,5],[6,7]].

On-device layout: channel-major ("CT", [chan partitions, token free]) for
q/k/v/o/res; token-major transient tiles (via PE transpose) only for the
kv_state contraction over tokens.  All matmuls in bf16 (fp32 PSUM accum).

Host pre-work (part of sharding): transpose+cast x to bf16 channel-major
with a 1-token halo on each side (for the depthwise conv), replicate
sin/cos across the 4 heads of a 128-channel tile, pre-transpose/cast all
weights into lhsT layout, build the rotate-every-two block matrix, head
mask and identity constants.
"""

import os
import sys

sys.path.insert(0, "/opt/trn_rl_repo")

import numpy as np
import ml_dtypes

B, N, DIM, H, HD = 4, 8192, 256, 8, 32
INTERNAL = H * HD  # 256
SCALE = HD ** -0.5
NCORES = 8
T = N // 2          # tokens per core
TH = T + 2          # with 1-token halo each side
CH = 512            # chunk tokens
NCH = T // CH       # chunks per core
KSC = SCALE / N     # kv_state scale (s^2)

BF16 = ml_dtypes.bfloat16


# ---------------------------------------------------------------- host prep

def _host_prep(x, sin, cos, W_qkvo, b_qkvo, W_lepe, b_lepe, W_proj, b_proj):
    """Build per-core input dicts (all device tensors)."""
    WT = W_qkvo.T.astype(np.float32)          # [DIM, 1024] = lhsT layout
    wq = WT[:, 0:256].astype(BF16)
    wkv = WT[:, 256:768].astype(BF16)          # k cols 0:256, v cols 256:512
    wo = WT[:, 768:1024].astype(BF16)
    wp = W_proj.T.astype(np.float32).astype(BF16)   # [DIM, 256] rhs layout
    wl = W_lepe[:, 0, :].astype(np.float32)    # [256, 3]

    # diag conv weights: block (tap j, tile m) = diag(wl[128m:128(m+1), j])
    dcw = np.zeros((128, 6, 128), np.float32)
    for j in range(3):
        for m in range(2):
            np.fill_diagonal(dcw[:, j * 2 + m, :], wl[128 * m:128 * (m + 1), j])
    dcw = dcw.reshape(128, 768).astype(BF16)

    # rotate-every-two matrix as lhsT: rot = R.T @ x ; R[k, m] = coeff of
    # chan k in rot-chan m:  rot[2i] = -x[2i+1], rot[2i+1] = x[2i]
    R = np.zeros((128, 128), np.float32)
    for i in range(64):
        R[2 * i + 1, 2 * i] = -1.0
        R[2 * i, 2 * i + 1] = 1.0
    R = R.astype(BF16)

    hmask = np.zeros((128, 128), np.float32)
    for hh in range(4):
        hmask[32 * hh:32 * (hh + 1), 32 * hh:32 * (hh + 1)] = 1.0
    hmask = hmask.astype(BF16)

    ident16 = np.eye(128, dtype=np.float32).astype(BF16)
    ident32 = np.eye(128, dtype=np.float32)

    use_bias = bool(np.any(b_qkvo) or np.any(b_lepe) or np.any(b_proj))
    bqkvo = np.asarray(b_qkvo, np.float32).reshape(1, 1024).astype(BF16)
    blep = np.asarray(b_lepe, np.float32).reshape(1, 256).astype(BF16)
    bprj = np.asarray(b_proj, np.float32).reshape(1, 256).astype(BF16)

    xf = np.asarray(x, np.float32)
    sinf = np.asarray(sin, np.float32)
    cosf = np.asarray(cos, np.float32)

    per_core = []
    for c in range(NCORES):
        b = c // 2
        t0 = (c % 2) * T
        # x channel-major with halo [256, TH]
        xpad = np.zeros((TH, DIM), np.float32)
        lo, hi = t0 - 1, t0 + T + 1
        slo, shi = max(lo, 0), min(hi, N)
        xpad[slo - lo: slo - lo + (shi - slo)] = xf[b, slo:shi]
        xct = np.ascontiguousarray(xpad.T).astype(BF16)          # [256, TH]

        srep = np.tile(sinf[t0:t0 + T].T, (4, 1)).astype(BF16)   # [128, T]
        crep = np.tile(cosf[t0:t0 + T].T, (4, 1)).astype(BF16)   # [128, T]
        # paired layout: col = c*1024 + j*512 + t, same data for j=0,1
        srp = np.ascontiguousarray(np.broadcast_to(
            srep.reshape(128, NCH, 1, CH), (128, NCH, 2, CH)).reshape(128, 2 * T))
        crp = np.ascontiguousarray(np.broadcast_to(
            crep.reshape(128, NCH, 1, CH), (128, NCH, 2, CH)).reshape(128, 2 * T))

        per_core.append({
            "xct": xct, "srep": srep, "crep": crep, "srp": srp, "crp": crp,
            "wq": np.ascontiguousarray(wq), "wkv": np.ascontiguousarray(wkv),
            "wo": np.ascontiguousarray(wo), "wp": np.ascontiguousarray(wp),
            "dcw": dcw, "rblk": R, "hmask": hmask,
            "ident16": ident16, "ident32": ident32,
            "bqkvo": bqkvo, "blep": blep, "bprj": bprj,
        })
    return per_core, use_bias


# ------------------------------------------------------------ device kernel

def _build_nc(use_bias: bool, lvl: int = 3):
    from concourse import bacc
    import concourse.mybir as mybir
    import concourse.tile as tile

    dt = mybir.dt
    AF = mybir.ActivationFunctionType
    OP = mybir.AluOpType

    nc = bacc.Bacc(None, target_bir_lowering=False)

    # ---- I/O
    xct_d = nc.dram_tensor("xct", [256, TH], dt.bfloat16, kind="ExternalInput")
    # paired sin/cos, col = c*1024 + j*512 + t (chunk-interleaved for both tiles)
    srp_d = nc.dram_tensor("srp", [128, 2 * T], dt.bfloat16, kind="ExternalInput")
    crp_d = nc.dram_tensor("crp", [128, 2 * T], dt.bfloat16, kind="ExternalInput")
    wq_d = nc.dram_tensor("wq", [256, 256], dt.bfloat16, kind="ExternalInput")
    wkv_d = nc.dram_tensor("wkv", [256, 512], dt.bfloat16, kind="ExternalInput")
    wo_d = nc.dram_tensor("wo", [256, 256], dt.bfloat16, kind="ExternalInput")
    wp_d = nc.dram_tensor("wp", [256, 256], dt.bfloat16, kind="ExternalInput")
    dcw_d = nc.dram_tensor("dcw", [128, 768], dt.bfloat16, kind="ExternalInput")
    rblk_d = nc.dram_tensor("rblk", [128, 128], dt.bfloat16, kind="ExternalInput")
    hmask_d = nc.dram_tensor("hmask", [128, 128], dt.bfloat16, kind="ExternalInput")
    id16_d = nc.dram_tensor("ident16", [128, 128], dt.bfloat16, kind="ExternalInput")
    id32_d = nc.dram_tensor("ident32", [128, 128], dt.float32, kind="ExternalInput")
    bqkvo_d = nc.dram_tensor("bqkvo", [1, 1024], dt.bfloat16, kind="ExternalInput")
    blep_d = nc.dram_tensor("blep", [1, 256], dt.bfloat16, kind="ExternalInput")
    bprj_d = nc.dram_tensor("bprj", [1, 256], dt.bfloat16, kind="ExternalInput")
    out_d = nc.dram_tensor("out", [T, 256], dt.float32, kind="ExternalOutput")

    RG = [[0, 1], [2, 3], [4, 5], [6, 7]]
    P2 = 2 * CH  # paired free size 1024

    with tile.TileContext(nc) as tc:
        with (
            tc.tile_pool(name="const", bufs=1) as const,
            tc.tile_pool(name="work", bufs=2) as work,
            tc.tile_pool(name="psum", bufs=2, space="PSUM") as ppool,
            tc.tile_pool(name="pacc", bufs=1, space="PSUM") as pacc,
            tc.tile_pool(name="dram", bufs=1, space="DRAM") as dpool,
        ):
            def load(tname, dten, shape, dtype=dt.bfloat16):
                t_ = const.tile(shape, dtype, tag=tname, name=tname)
                nc.sync.dma_start(out=t_, in_=dten[:, :])
                return t_

            xct = [const.tile([128, TH], dt.bfloat16, tag=f"xct{k}", name=f"xct{k}")
                   for k in range(2)]
            for k in range(2):
                nc.sync.dma_start(out=xct[k], in_=xct_d[128 * k:128 * (k + 1), :])
            srp = load("srp", srp_d, [128, 2 * T])
            crp = load("crp", crp_d, [128, 2 * T])
            wq = [const.tile([128, 256], dt.bfloat16, tag=f"wq{k}", name=f"wq{k}")
                  for k in range(2)]
            wkv = [const.tile([128, 512], dt.bfloat16, tag=f"wkv{k}", name=f"wkv{k}")
                   for k in range(2)]
            wo = [const.tile([128, 256], dt.bfloat16, tag=f"wo{k}", name=f"wo{k}")
                  for k in range(2)]
            wp = [const.tile([128, 256], dt.bfloat16, tag=f"wp{k}", name=f"wp{k}")
                  for k in range(2)]
            for k in range(2):
                sl = slice(128 * k, 128 * (k + 1))
                nc.sync.dma_start(out=wq[k], in_=wq_d[sl, :])
                nc.sync.dma_start(out=wkv[k], in_=wkv_d[sl, :])
                nc.sync.dma_start(out=wo[k], in_=wo_d[sl, :])
                nc.sync.dma_start(out=wp[k], in_=wp_d[sl, :])
            dcw = load("dcw", dcw_d, [128, 768])
            rblk = load("rblk", rblk_d, [128, 128])
            hmask = load("hmask", hmask_d, [128, 128])
            id16 = load("id16", id16_d, [128, 128])
            id32 = load("id32", id32_d, [128, 128], dt.float32)
            ones = None
            if use_bias:
                bqkvo = load("bqkvo", bqkvo_d, [1, 1024])
                blep = load("blep", blep_d, [1, 256])
                bprj = load("bprj", bprj_d, [1, 256])
                ones = const.tile([1, CH], dt.bfloat16, tag="ones", name="ones")
                nc.vector.memset(ones, 1.0)

            # persistent activations (paired layout: col c*1024 + j*512 + t)
            q1p = const.tile([128, 2 * T], dt.bfloat16, tag="q1p", name="q1p")
            o1p = const.tile([128, 2 * T], dt.bfloat16, tag="o1p", name="o1p")
            vT = [const.tile([128, TH], dt.bfloat16, tag=f"vT{j}", name=f"vT{j}")
                  for j in range(2)]
            kpart = const.tile([128, 32], dt.float32, tag="kpart", name="kpart")
            vpart = const.tile([128, 16], dt.float32, tag="vpart", name="vpart")
            stats = const.tile([128, 260], dt.float32, tag="stats", name="stats")
            stats2 = const.tile([128, 260], dt.float32, tag="stats2", name="stats2")

            gram = pacc.tile([128, 256], dt.float32, tag="gram", name="gram")

            # =========================== phase 1 ===========================
            for c in range(NCH):
                xsl = [x[:, 1 + c * CH: 1 + (c + 1) * CH] for x in xct]
                psl = slice(c * P2, (c + 1) * P2)       # paired persists slice
                ssl = srp[:, psl]
                csl = crp[:, psl]

                # ---- q / k / v / o projections (paired psum, per-bank groups)
                qps = ppool.tile([128, P2], dt.float32, tag="w", name="qps")
                kps = ppool.tile([128, P2], dt.float32, tag="w", name="kps")
                vps = ppool.tile([128, P2], dt.float32, tag="w", name="vps")
                ops_ = ppool.tile([128, P2], dt.float32, tag="w", name="ops_")
                for j in range(2):
                    cols = slice(j * CH, (j + 1) * CH)
                    msl = slice(128 * j, 128 * (j + 1))
                    nc.tensor.matmul(qps[:, cols], wq[0][:, msl], xsl[0],
                                     start=True, stop=False)
                    nc.tensor.matmul(qps[:, cols], wq[1][:, msl], xsl[1],
                                     start=False, stop=not use_bias)
                    if use_bias:
                        nc.tensor.matmul(qps[:, cols], bqkvo[:, msl], ones,
                                         start=False, stop=True)
                    nc.tensor.matmul(kps[:, cols], wkv[0][:, msl], xsl[0],
                                     start=True, stop=False)
                    nc.tensor.matmul(kps[:, cols], wkv[1][:, msl], xsl[1],
                                     start=False, stop=not use_bias)
                    if use_bias:
                        nc.tensor.matmul(kps[:, cols],
                                         bqkvo[:, 256 + 128 * j:256 + 128 * (j + 1)],
                                         ones, start=False, stop=True)
                    vsl = slice(256 + 128 * j, 256 + 128 * (j + 1))
                    nc.tensor.matmul(vps[:, cols], wkv[0][:, vsl], xsl[0],
                                     start=True, stop=False)
                    nc.tensor.matmul(vps[:, cols], wkv[1][:, vsl], xsl[1],
                                     start=False, stop=not use_bias)
                    if use_bias:
                        nc.tensor.matmul(vps[:, cols],
                                         bqkvo[:, 512 + 128 * j:512 + 128 * (j + 1)],
                                         ones, start=False, stop=True)
                    nc.tensor.matmul(ops_[:, cols], wo[0][:, msl], xsl[0],
                                     start=True, stop=False)
                    nc.tensor.matmul(ops_[:, cols], wo[1][:, msl], xsl[1],
                                     start=False, stop=not use_bias)
                    if use_bias:
                        nc.tensor.matmul(ops_[:, cols],
                                         bqkvo[:, 768 + 128 * j:768 + 128 * (j + 1)],
                                         ones, start=False, stop=True)

                # ---- q elu+1 -> q1p (paired ops)
                rq = work.tile([128, P2], dt.bfloat16, tag="rq", name="rq")
                nc.scalar.activation(rq, qps, AF.Relu)
                mn = work.tile([128, P2], dt.bfloat16, tag="mn", name="mn")
                nc.vector.tensor_scalar_min(mn, qps, 0.0)
                eq = work.tile([128, P2], dt.bfloat16, tag="eq", name="eq")
                nc.scalar.activation(eq, mn, AF.Exp)
                nc.gpsimd.tensor_add(q1p[:, psl], eq, rq)

                # ---- o evac -> o1p
                nc.scalar.activation(o1p[:, psl], ops_, AF.Copy)

                # ---- k elu+1 (ksum rides ACT accums)
                rk = work.tile([128, P2], dt.bfloat16, tag="rk", name="rk")
                nc.scalar.activation(rk[:, 0:CH], kps[:, 0:CH], AF.Relu,
                                     accum_out=kpart[:, c:c + 1])
                nc.scalar.activation(rk[:, CH:P2], kps[:, CH:P2], AF.Relu,
                                     accum_out=kpart[:, 8 + c:9 + c])
                mnk = work.tile([128, P2], dt.bfloat16, tag="mnk", name="mnk")
                nc.vector.tensor_scalar_min(mnk, kps, 0.0)
                ek = work.tile([128, P2], dt.bfloat16, tag="ek", name="ek")
                nc.scalar.activation(ek[:, 0:CH], mnk[:, 0:CH], AF.Exp,
                                     accum_out=kpart[:, 16 + c:17 + c])
                nc.scalar.activation(ek[:, CH:P2], mnk[:, CH:P2], AF.Exp,
                                     accum_out=kpart[:, 24 + c:25 + c])
                k1t = work.tile([128, P2], dt.bfloat16, tag="k1t", name="k1t")
                nc.vector.tensor_add(k1t, ek, rk)

                # ---- v evac -> vT (vsum rides ACT accums)
                for j in range(2):
                    nc.scalar.activation(
                        vT[j][:, 1 + c * CH: 1 + (c + 1) * CH],
                        vps[:, j * CH:(j + 1) * CH], AF.Copy,
                        accum_out=vpart[:, 8 * j + c: 8 * j + c + 1])

                # ---- K rope (one paired rot MM)
                rkp = ppool.tile([128, P2], dt.float32, tag="w", name="rkp")
                nc.tensor.matmul(rkp[:, 0:CH], rblk, k1t[:, 0:CH],
                                 start=True, stop=True)
                nc.tensor.matmul(rkp[:, CH:P2], rblk, k1t[:, CH:P2],
                                 start=True, stop=True)
                m1 = work.tile([128, P2], dt.bfloat16, tag="m1", name="m1")
                nc.gpsimd.tensor_mul(m1, k1t, csl)
                m2 = work.tile([128, P2], dt.bfloat16, tag="m2", name="m2")
                nc.vector.tensor_mul(m2, rkp, ssl)
                ks = work.tile([128, P2], dt.bfloat16, tag="ks", name="ks")
                nc.vector.tensor_add(ks, m1, m2)

                # ---- transpose ks, v to token-major; kv gram accumulation
                for s in range(4):
                    ktp = ppool.tile([128, CH], dt.bfloat16, tag="tp", bufs=2,
                                     name="ktp")
                    nc.tensor.transpose(ktp[:, 0:128],
                                        ks[:, s * 128:(s + 1) * 128], id16)
                    nc.tensor.transpose(ktp[:, 128:256],
                                        ks[:, CH + s * 128:CH + (s + 1) * 128], id16)
                    vcol = 1 + c * CH + s * 128
                    nc.tensor.transpose(ktp[:, 256:384],
                                        vT[0][:, vcol:vcol + 128], id16)
                    nc.tensor.transpose(ktp[:, 384:512],
                                        vT[1][:, vcol:vcol + 128], id16)
                    kvtok = work.tile([128, CH], dt.bfloat16, tag="kvtok",
                                      name="kvtok")
                    if s % 2 == 0:
                        nc.scalar.activation(kvtok, ktp, AF.Copy)
                    else:
                        nc.vector.tensor_copy(kvtok, ktp)
                    first = (c == 0 and s == 0)
                    last = (c == NCH - 1 and s == 3)
                    nc.tensor.matmul(gram[:, 0:128], kvtok[:, 0:128],
                                     kvtok[:, 256:384], start=first, stop=False)
                    nc.tensor.matmul(gram[:, 128:256], kvtok[:, 128:256],
                                     kvtok[:, 384:512], start=False, stop=last)

            # ---- halo v columns (tokens t0-1 and t0+T) for the conv
            vhp = ppool.tile([128, CH], dt.float32, tag="tp", bufs=2, name="vhp")
            for j in range(2):
                vsl = slice(256 + 128 * j, 256 + 128 * (j + 1))
                cl = slice(j * 4, j * 4 + 1)
                cr = slice(j * 4 + 2, j * 4 + 3)
                nc.tensor.matmul(vhp[:, cl], wkv[0][:, vsl], xct[0][:, 0:1],
                                 start=(j == 0), stop=False)
                nc.tensor.matmul(vhp[:, cl], wkv[1][:, vsl], xct[1][:, 0:1],
                                 start=False, stop=False)
                nc.tensor.matmul(vhp[:, cr], wkv[0][:, vsl], xct[0][:, TH - 1:TH],
                                 start=False, stop=False)
                nc.tensor.matmul(vhp[:, cr], wkv[1][:, vsl], xct[1][:, TH - 1:TH],
                                 start=False, stop=(j == 1))
            for j in range(2):
                nc.scalar.activation(vT[j][:, 0:1], vhp[:, j * 4:j * 4 + 1], AF.Copy)
                nc.scalar.activation(vT[j][:, TH - 1:TH],
                                     vhp[:, j * 4 + 2:j * 4 + 3], AF.Copy)

            # ======================= stats + allreduce =====================
            nc.vector.tensor_scalar_mul(stats[:, 0:256], gram, 1.0)
            nc.vector.tensor_reduce(stats[:, 256:257], kpart[:, 0:8],
                                    axis=mybir.AxisListType.X, op=OP.add)
            nc.vector.tensor_reduce(stats[:, 257:258], kpart[:, 8:16],
                                    axis=mybir.AxisListType.X, op=OP.add)
            kx = const.tile([128, 2], dt.float32, tag="kx", name="kx")
            nc.vector.tensor_reduce(kx[:, 0:1], kpart[:, 16:24],
                                    axis=mybir.AxisListType.X, op=OP.add)
            nc.vector.tensor_reduce(kx[:, 1:2], kpart[:, 24:32],
                                    axis=mybir.AxisListType.X, op=OP.add)
            nc.vector.tensor_add(stats[:, 256:258], stats[:, 256:258], kx)
            nc.vector.tensor_reduce(stats[:, 258:259], vpart[:, 0:8],
                                    axis=mybir.AxisListType.X, op=OP.add)
            nc.vector.tensor_reduce(stats[:, 259:260], vpart[:, 8:16],
                                    axis=mybir.AxisListType.X, op=OP.add)

            if lvl >= 2 and os.environ.get("KERNEL_NOCC"):
                nc.vector.tensor_scalar_mul(stats2, stats, 1.0)
            elif lvl >= 2:
                ccin = dpool.tile([128, 260], dt.float32, tag="ccin", name="ccin")
                ccout = dpool.tile([128, 260], dt.float32, tag="ccout", name="ccout")
                nc.gpsimd.dma_start(out=ccin[:, :], in_=stats)
                nc.gpsimd.collective_compute(
                    "AllReduce", OP.add, replica_groups=RG,
                    ins=[ccin[:, :]], outs=[ccout[:, :]])
                nc.gpsimd.dma_start(out=stats2, in_=ccout[:, :])
            else:
                nc.vector.tensor_scalar_mul(stats2, stats, 1.0)

            if lvl <= 2:
                for c in range(NCH):
                    dummy = work.tile([128, CH], dt.float32, tag="outsb",
                                      name="dummy")
                    nc.vector.tensor_scalar_mul(
                        dummy, stats2[:, 0:1].to_broadcast((128, CH)), 1.0)
                    for h in range(2):
                        dsl = out_d[c * CH + h * 256: c * CH + (h + 1) * 256, :]
                        nc.sync.dma_start(
                            out=dsl.rearrange("(s t) o -> t s o", s=2), in_=dummy)

            if lvl >= 3:
                _phase2(locals())

    nc.compile()
    return nc


def _phase2(env):
    nc = env["nc"]; dt = env["dt"]; AF = env["AF"]; OP = env["OP"]
    const = env["const"]; work = env["work"]; ppool = env["ppool"]
    use_bias = env["use_bias"]; stats2 = env["stats2"]; hmask = env["hmask"]
    id32 = env["id32"]; srp = env["srp"]; crp = env["crp"]
    wp = env["wp"]; dcw = env["dcw"]; rblk = env["rblk"]
    q1p = env["q1p"]; o1p = env["o1p"]; vT = env["vT"]; out_d = env["out_d"]
    ones = env["ones"]
    P2 = 2 * CH
    if use_bias:
        blep = env["blep"]; bprj = env["bprj"]
    import concourse.mybir as mybir

    # ---- assemble small matrices
    zsc = const.tile([128, 2], dt.float32, tag="zsc", name="zsc")
    nc.scalar.mul(zsc[:, 0:1], stats2[:, 256:257], SCALE / N)
    nc.scalar.mul(zsc[:, 1:2], stats2[:, 257:258], SCALE / N)

    zblk = []
    mcorr = []
    kvblk = []
    for j in range(2):
        zb = const.tile([128, 128], dt.bfloat16, tag=f"zblk{j}", name=f"zblk{j}")
        nc.vector.tensor_tensor(
            zb, zsc[:, j:j + 1].to_broadcast((128, 128)), hmask, OP.mult)
        zblk.append(zb)

        vrp = ppool.tile([128, CH], dt.float32, tag="tp", bufs=2, name="vrp")
        nc.tensor.transpose(vrp[0:1, 0:128], stats2[:, 258 + j:259 + j], id32)
        vrow = const.tile([1, 128], dt.float32, tag=f"vrow{j}", name=f"vrow{j}")
        nc.scalar.mul(vrow, vrp[0:1, 0:128], -1.0 / N)
        vrowb = const.tile([128, 128], dt.float32, tag=f"vrowb{j}", name=f"vrowb{j}")
        nc.gpsimd.partition_broadcast(vrowb, vrow)
        mc0 = const.tile([128, 128], dt.float32, tag=f"mc0{j}", name=f"mc0{j}")
        nc.vector.tensor_tensor(
            mc0, zsc[:, j:j + 1].to_broadcast((128, 128)), vrowb, OP.mult)
        mc = const.tile([128, 128], dt.bfloat16, tag=f"mc{j}", name=f"mc{j}")
        nc.vector.tensor_tensor(mc, mc0, hmask, OP.mult)
        mcorr.append(mc)

        kvb = const.tile([128, 128], dt.bfloat16, tag=f"kvb{j}", name=f"kvb{j}")
        nc.vector.memset(kvb, 0.0)
        for a in range(4):
            psl = slice(32 * a, 32 * (a + 1))
            nc.scalar.mul(kvb[psl, psl],
                          stats2[psl, 128 * j + 32 * a: 128 * j + 32 * (a + 1)],
                          KSC)
        kvblk.append(kvb)

    # =========================== phase 2 ===========================
    for c in range(NCH):
        psl = slice(c * P2, (c + 1) * P2)
        ssl = srp[:, psl]
        csl = crp[:, psl]
        q1sl = q1p[:, psl]

        zps = ppool.tile([128, P2], dt.float32, tag="w", name="zps")
        nc.tensor.matmul(zps[:, 0:CH], zblk[0], q1p[:, c * P2:c * P2 + CH],
                         start=True, stop=True)
        nc.tensor.matmul(zps[:, CH:P2], zblk[1], q1p[:, c * P2 + CH:(c + 1) * P2],
                         start=True, stop=True)
        rz = work.tile([128, P2], dt.float32, tag="rz", name="rz")
        nc.vector.reciprocal_approx_fast(out=rz, in_=zps)
        qa = work.tile([128, P2], dt.bfloat16, tag="qa", name="qa")
        nc.vector.scalar_tensor_tensor(out=qa, in0=rz, scalar=1.0, in1=q1sl,
                                       op0=OP.add, op1=OP.mult)
        rqp = ppool.tile([128, P2], dt.float32, tag="w", name="rqp")
        nc.tensor.matmul(rqp[:, 0:CH], rblk, qa[:, 0:CH], start=True, stop=True)
        nc.tensor.matmul(rqp[:, CH:P2], rblk, qa[:, CH:P2], start=True, stop=True)
        t1 = work.tile([128, P2], dt.bfloat16, tag="t1", name="t1")
        nc.gpsimd.tensor_mul(t1, qa, csl)
        t2 = work.tile([128, P2], dt.bfloat16, tag="t2", name="t2")
        nc.vector.tensor_mul(t2, rqp, ssl)

        rps = ppool.tile([128, P2], dt.float32, tag="w", name="rps")
        for j in range(2):
            cols = slice(j * CH, (j + 1) * CH)
            nc.tensor.matmul(rps[:, cols], kvblk[j], t1[:, cols],
                             start=True, stop=False)
            nc.tensor.matmul(rps[:, cols], kvblk[j], t2[:, cols],
                             start=False, stop=False)
            nc.tensor.matmul(rps[:, cols], mcorr[j],
                             q1p[:, c * P2 + j * CH: c * P2 + (j + 1) * CH],
                             start=False, stop=False)
            for tap in range(3):
                lastmm = (tap == 2 and not use_bias)
                nc.tensor.matmul(
                    rps[:, cols],
                    dcw[:, (tap * 2 + j) * 128:(tap * 2 + j + 1) * 128],
                    vT[j][:, c * CH + tap: c * CH + tap + CH],
                    start=False, stop=lastmm)
            if use_bias:
                nc.tensor.matmul(rps[:, cols], blep[:, 128 * j:128 * (j + 1)],
                                 ones, start=False, stop=True)

        y = work.tile([128, P2], dt.bfloat16, tag="y", name="y")
        nc.vector.tensor_mul(y, rps, o1p[:, psl])

        outp = ppool.tile([128, P2], dt.float32, tag="w", name="outp")
        for h in range(2):
            for si in range(2):
                s = h * 2 + si
                osl = slice(s * 256, (s + 1) * 256)
                first = (si == 0)
                nc.tensor.matmul(outp[:, osl], y[:, s * 128:(s + 1) * 128],
                                 wp[0], start=first, stop=False)
                last = (si == 1 and not use_bias)
                nc.tensor.matmul(outp[:, osl],
                                 y[:, CH + s * 128:CH + (s + 1) * 128],
                                 wp[1], start=False, stop=last)
                if use_bias:
                    nc.tensor.matmul(outp[:, osl], ones[:, 0:128], bprj,
                                     start=False, stop=(si == 1))
        outsb = work.tile([128, P2], dt.float32, tag="outsb", name="outsb")
        nc.scalar.activation(outsb, outp, AF.Copy)
        dsl = out_d[c * CH: (c + 1) * CH, :]
        nc.sync.dma_start(out=dsl.rearrange("(s t) o -> t s o", s=4), in_=outsb)



_NC_CACHE = {}


def _get_nc(use_bias: bool):
    lvl = int(os.environ.get("KERNEL_LVL", "3"))
    key = (use_bias, lvl)
    if key not in _NC_CACHE:
        _NC_CACHE[key] = _build_nc(use_bias, lvl)
    return _NC_CACHE[key]


def kernel(x, sin, cos, W_qkvo, b_qkvo, W_lepe, b_lepe, W_proj, b_proj):
    from concourse.bass_utils import run_bass_kernel_spmd

    per_core, use_bias = _host_prep(x, sin, cos, W_qkvo, b_qkvo, W_lepe,
                                    b_lepe, W_proj, b_proj)
    nc = _get_nc(use_bias)
    # keep only the inputs that survived DCE in the compiled program
    import concourse.mybir as mybir
    expected = set()
    for alloc in nc.m.functions[0].allocations:
        if isinstance(alloc, mybir.MemoryLocationSet) and alloc.kind == "ExternalInput":
            expected.add(alloc.memorylocations[0].name)
    per_core = [{k: v for k, v in m.items() if k in expected} for m in per_core]
    res = run_bass_kernel_spmd(nc, per_core, core_ids=list(range(NCORES)),
                               trace=bool(os.environ.get("KERNEL_TRACE")))
    if os.environ.get("KERNEL_TRACE"):
        kernel.last_exec_time_ns = res.exec_time_ns
        kernel.last_results = res
    full = np.zeros((B, N, INTERNAL), np.float32)
    for c in range(NCORES):
        b = c // 2
        t0 = (c % 2) * T
        full[b, t0:t0 + T] = res.results[c]["out"]
    return full


# ---------------------------------------------------------- numpy reference
# A numpy emulation of the exact device pipeline (fp32), used to validate
# the decomposition (run with KERNEL_SELFTEST=1).

def _numpy_pipeline(per_core_inputs, skip_pair=False):
    outs = []
    cores = []
    for c in range(NCORES):
        d = per_core_inputs[c]
        xct = d["xct"].astype(np.float32)          # [256, TH]
        srep = d["srep"].astype(np.float32)
        crep = d["crep"].astype(np.float32)
        wq = d["wq"].astype(np.float32)
        wkv = d["wkv"].astype(np.float32)
        wo = d["wo"].astype(np.float32)
        wp = d["wp"].astype(np.float32)
        dcw = d["dcw"].astype(np.float32).reshape(128, 6, 128)
        R = d["rblk"].astype(np.float32)
        hmask = d["hmask"].astype(np.float32)

        x_in = xct[:, 1:T + 1]                     # [256, T]
        qT = wq.T @ x_in                           # [256, T]
        kT = wkv[:, 0:256].T @ x_in
        vT_m = wkv[:, 256:512].T @ x_in
        oT = wo.T @ x_in
        # halo v cols
        vhl = wkv[:, 256:512].T @ xct[:, 0:1]
        vhr = wkv[:, 256:512].T @ xct[:, TH - 1:TH]
        vT = np.concatenate([vhl, vT_m, vhr], axis=1)      # [256, TH]

        def elu1(t):
            return np.exp(np.minimum(t, 0.0)) + np.maximum(t, 0.0)

        q1 = elu1(qT)
        k1 = elu1(kT)

        # K rope (per chan-tile with R)
        ks = np.zeros_like(k1)
        for j in range(2):
            blk = k1[128 * j:128 * (j + 1)]
            ks[128 * j:128 * (j + 1)] = blk * crep + (R.T @ blk) * srep

        # kv gram per tile: ks_j^T tokens x v_j
        gram = np.zeros((128, 256), np.float32)
        for j in range(2):
            gram[:, 128 * j:128 * (j + 1)] = (
                ks[128 * j:128 * (j + 1)] @ vT[128 * j:128 * (j + 1), 1:T + 1].T)
        ksum = k1.sum(axis=1)                      # [256]
        vsum = vT[:, 1:T + 1].sum(axis=1)
        cores.append(dict(d=d, q1=q1, oT=oT, vT=vT, gram=gram, ksum=ksum,
                          vsum=vsum, R=R, hmask=hmask, dcw=dcw, wp=wp,
                          srep=srep, crep=crep))

    for pair in range(4):
        a, b2 = cores[2 * pair], cores[2 * pair + 1]
        if skip_pair:
            for cc in (a, b2):
                cc["gram_r"], cc["ksum_r"], cc["vsum_r"] = (
                    cc["gram"], cc["ksum"], cc["vsum"])
            continue
        gram = a["gram"] + b2["gram"]
        ksum = a["ksum"] + b2["ksum"]
        vsum = a["vsum"] + b2["vsum"]
        for cc in (a, b2):
            cc["gram_r"], cc["ksum_r"], cc["vsum_r"] = gram, ksum, vsum

    for c in range(NCORES):
        st = cores[c]
        q1, oT, vT = st["q1"], st["oT"], st["vT"]
        R, hmask, dcw, wp = st["R"], st["hmask"], st["dcw"], st["wp"]
        srep, crep = st["srep"], st["crep"]
        gram, ksum, vsum = st["gram_r"], st["ksum_r"], st["vsum_r"]

        kmean = ksum / N
        vmean = vsum / N
        out = np.zeros((T, 256), np.float32)
        res = np.zeros((256, T), np.float32)
        for j in range(2):
            sl = slice(128 * j, 128 * (j + 1))
            zsc = SCALE * kmean[sl]                          # [128]
            zblk = (zsc[:, None] * hmask)                    # [128,128]
            zrep = zblk.T @ q1[sl]                           # [128, T]
            r = 1.0 / zrep
            qa = q1[sl] * (1.0 + r)
            t1 = qa * crep
            t2 = (R.T @ qa) * srep
            kvblk = np.zeros((128, 128), np.float32)
            for aa in range(4):
                s2 = slice(32 * aa, 32 * (aa + 1))
                kvblk[s2, s2] = KSC * gram[s2, 128 * j + 32 * aa:128 * j + 32 * (aa + 1)]
            mcorr = -(zsc[:, None]) * (vmean[sl][None, :] / 1.0) * hmask / 1.0
            mcorr = mcorr * 1.0
            # note: corr = z (x) vmean -> M[k, c] = SCALE*kmean[k]*vmean[c]*mask
            lepe = np.zeros((128, T), np.float32)
            for tap in range(3):
                dw = dcw[:, tap * 2 + j, :]
                lepe += dw.T @ vT[sl, tap:tap + T]
            res[sl] = (kvblk.T @ t1 + kvblk.T @ t2 + mcorr.T @ q1[sl] + lepe)
        y = res * oT
        out = y.T @ wp            # wait: out[t, oc] = sum_c y[c,t] wp[c,oc]
        outs.append(out.astype(np.float32))

    # unshard
    full = np.zeros((B, N, 256), np.float32)
    for c in range(NCORES):
        b = c // 2
        t0 = (c % 2) * T
        full[b, t0:t0 + T] = outs[c]
    return full


if __name__ == "__main__" and os.environ.get("KERNEL_BUILD"):
    nc = _build_nc(False)
    import tempfile
    from concourse.bass_utils import compile_bass_kernel
    print("NEFF:", compile_bass_kernel(nc, tempfile.mkdtemp()))

if __name__ == "__main__" and os.environ.get("KERNEL_SELFTEST"):
    sys.path.insert(0, os.path.dirname(os.path.abspath(__file__)))
    import reference
    inputs = reference.setup_inputs()
    inputs = {k: np.asarray(v) for k, v in inputs.items()}
    expected = np.asarray(reference.reference(**inputs))
    per_core, use_bias = _host_prep(**inputs)
    got = _numpy_pipeline(per_core)
    err = np.abs(got - expected)
    rel = np.linalg.norm(got - expected) / np.linalg.norm(expected)
    print("selftest rel err:", rel, "max abs:", err.max())



# revision 9
# speedup vs baseline: 1.3229x; 1.3229x over previous
"""Trainium2 Bass kernel for nn_MultiHeadMALAAttention.

Sharding (tensor-parallel over heads): 8 cores; core c handles batch
b = c//2 and head-group hg = c%2 (4 heads = 128 internal channels) over
ALL N=8192 tokens.  kv_state / ksum / vsum / z are per-head -> fully
core-local, no collective.  The output projection is a partial sum over
the core's 128 channels; the host adds the two partials of each batch.

Device pipeline per core:
  Phase A (per 512-token chunk): k,v projected TOKEN-major directly
    (lhsT = x channel-major slice, rhs = W) -> elu(k)+1 via
    min(exp,1)+relu -> rope folded into the kv gram: accumulate
    gram_a += (k1*cos)^T v and gram_b += (k1*sswap)^T v, where sswap is
    the pair-swapped+signed sin (host precomputed); after the loop
    gram = gram_a + P @ gram_b with P the pair-swap permutation (one
    matmul).  v and o are also produced channel-major (v for the LEPE
    conv + vsum, o for the phase-2 gate).  ksum via ones-matmul on k1.
  Phase 2 (per 512-token chunk): q projected channel-major; elu; rope
    of q via the R block matmul; z = zblk^T q1; attn = kvb^T qs;
    rest = mcorr^T q1 + depthwise conv taps (diag matmuls);
    y = (attn*(1+1/z) + rest) * o; out = wp^T y (channel-major bf16
    partials) -> DMA.  Host: transpose + add the two partials per batch.
"""

import os
import sys

sys.path.insert(0, "/opt/trn_rl_repo")

import numpy as np
import ml_dtypes

B, N, DIM, H, HD = 4, 8192, 256, 8, 32
INTERNAL = H * HD  # 256
SCALE = HD ** -0.5
NCORES = 8

CH = 512            # chunk tokens
NCH = N // CH       # 16 chunks per core
TH = N + 2          # vT with 1-token zero halo each side

BF16 = ml_dtypes.bfloat16


# ---------------------------------------------------------------- host prep

def _host_prep(x, sin, cos, W_qkvo, b_qkvo, W_lepe, b_lepe, W_proj, b_proj):
    """Build per-core input dicts (all device tensors)."""
    WT = np.asarray(W_qkvo, np.float32).T          # [DIM, 1024] lhsT layout
    WPT = np.asarray(W_proj, np.float32).T         # [INTERNAL, DIM]
    wl = np.asarray(W_lepe, np.float32)[:, 0, :]   # [256, 3]
    sinf = np.asarray(sin, np.float32)             # [N, 32]
    cosf = np.asarray(cos, np.float32)
    xf = np.asarray(x, np.float32)

    # R: rot = R.T @ x ; rot[2i] = -x[2i+1], rot[2i+1] = x[2i]
    R = np.zeros((128, 128), np.float32)
    for i in range(64):
        R[2 * i + 1, 2 * i] = -1.0
        R[2 * i, 2 * i + 1] = 1.0
    rblk = R.astype(BF16)

    # P: pair-swap permutation (symmetric)
    P = np.zeros((128, 128), np.float32)
    for i in range(64):
        P[2 * i, 2 * i + 1] = 1.0
        P[2 * i + 1, 2 * i] = 1.0
    pmat = P.astype(BF16)

    hmask = np.zeros((128, 128), np.float32)
    for hh in range(4):
        hmask[32 * hh:32 * (hh + 1), 32 * hh:32 * (hh + 1)] = 1.0
    hmask = hmask.astype(BF16)

    id32 = np.eye(128, dtype=np.float32)

    # token-major compact sin/cos for the k rope: [128, 32] per 128-token
    # tile -> [128, N/4].  stm is the swapped+signed sin:
    # sswap[t, 2i] = sin[t, 2i+1], sswap[t, 2i+1] = -sin[t, 2i]
    sswap = np.empty_like(sinf)
    sswap[:, 0::2] = sinf[:, 1::2]
    sswap[:, 1::2] = -sinf[:, 0::2]
    ntile = N // 128
    ctm = np.ascontiguousarray(
        cosf.reshape(ntile, 128, 32).transpose(1, 0, 2).reshape(128, N // 4)
    ).astype(BF16)
    stm = np.ascontiguousarray(
        sswap.reshape(ntile, 128, 32).transpose(1, 0, 2).reshape(128, N // 4)
    ).astype(BF16)

    # channel-major sin/cos for the q rope: [128, N], rows = 4 heads x 32
    ccm = np.ascontiguousarray(np.tile(cosf.T, (4, 1))).astype(BF16)
    scm = np.ascontiguousarray(np.tile(sinf.T, (4, 1))).astype(BF16)

    use_bias = bool(np.any(b_qkvo) or np.any(b_lepe) or np.any(b_proj))

    shared = {"rblk": rblk, "pmat": pmat, "hmask": hmask, "id32": id32,
              "ctm": ctm, "stm": stm, "ccm": ccm, "scm": scm}
    per_core = []
    xcts = {}
    for c in range(NCORES):
        b = c // 2
        hg = c % 2
        osl = slice(hg * 128, hg * 128 + 128)
        if b not in xcts:
            xcts[b] = np.ascontiguousarray(xf[b].T).astype(BF16)  # [256, N]
        # [dim, k-chans own | v-chans own]
        wkvtm = np.ascontiguousarray(np.concatenate(
            [WT[:, 256 + hg * 128:256 + hg * 128 + 128],
             WT[:, 512 + hg * 128:512 + hg * 128 + 128]], axis=1)).astype(BF16)
        wq = np.ascontiguousarray(WT[:, hg * 128:hg * 128 + 128]).astype(BF16)
        wo = np.ascontiguousarray(
            WT[:, 768 + hg * 128:768 + hg * 128 + 128]).astype(BF16)
        wp = np.ascontiguousarray(WPT[osl, :]).astype(BF16)       # [128, 256]

        wlo = wl[osl]                                             # [128, 3]
        dcw = np.zeros((128, 3, 128), np.float32)
        for tap in range(3):
            np.fill_diagonal(dcw[:, tap, :], wlo[:, tap])
        dcw = dcw.reshape(128, 384).astype(BF16)

        d = {"xct": xcts[b], "wkvtm": wkvtm, "wq": wq, "wo": wo, "wp": wp,
             "dcw": dcw}
        d.update(shared)
        if use_bias:
            bq = np.asarray(b_qkvo, np.float32)
            d["bkv"] = np.ascontiguousarray(np.concatenate(
                [bq[256 + hg * 128:256 + hg * 128 + 128],
                 bq[512 + hg * 128:512 + hg * 128 + 128]]
            ).reshape(1, 256)).astype(BF16)
            d["bq"] = np.ascontiguousarray(
                bq[hg * 128:hg * 128 + 128].reshape(1, 128)).astype(BF16)
            d["bo"] = np.ascontiguousarray(
                bq[768 + hg * 128:768 + hg * 128 + 128].reshape(1, 128)
            ).astype(BF16)
            d["blep"] = np.ascontiguousarray(
                np.asarray(b_lepe, np.float32)[osl].reshape(1, 128)).astype(BF16)
            d["bprj"] = np.ascontiguousarray(
                np.asarray(b_proj, np.float32).reshape(1, 256)).astype(BF16)
        per_core.append(d)
    return per_core, use_bias


# ------------------------------------------------------------ device kernel

def _build_nc(use_bias: bool, nch: int = NCH):
    from concourse import bacc
    import concourse.mybir as mybir
    import concourse.tile as tile

    dt = mybir.dt
    AF = mybir.ActivationFunctionType
    OP = mybir.AluOpType

    n_tok = nch * CH
    th = n_tok + 2

    nc = bacc.Bacc(None, target_bir_lowering=False)

    xct_d = nc.dram_tensor("xct", [256, n_tok], dt.bfloat16, kind="ExternalInput")
    wkvtm_d = nc.dram_tensor("wkvtm", [256, 256], dt.bfloat16, kind="ExternalInput")
    wq_d = nc.dram_tensor("wq", [256, 128], dt.bfloat16, kind="ExternalInput")
    wo_d = nc.dram_tensor("wo", [256, 128], dt.bfloat16, kind="ExternalInput")
    wp_d = nc.dram_tensor("wp", [128, 256], dt.bfloat16, kind="ExternalInput")
    dcw_d = nc.dram_tensor("dcw", [128, 384], dt.bfloat16, kind="ExternalInput")
    rblk_d = nc.dram_tensor("rblk", [128, 128], dt.bfloat16, kind="ExternalInput")
    pmat_d = nc.dram_tensor("pmat", [128, 128], dt.bfloat16, kind="ExternalInput")
    hmask_d = nc.dram_tensor("hmask", [128, 128], dt.bfloat16, kind="ExternalInput")
    id32_d = nc.dram_tensor("id32", [128, 128], dt.float32, kind="ExternalInput")
    ctm_d = nc.dram_tensor("ctm", [128, n_tok // 4], dt.bfloat16,
                           kind="ExternalInput")
    stm_d = nc.dram_tensor("stm", [128, n_tok // 4], dt.bfloat16,
                           kind="ExternalInput")
    ccm_d = nc.dram_tensor("ccm", [128, n_tok], dt.bfloat16, kind="ExternalInput")
    scm_d = nc.dram_tensor("scm", [128, n_tok], dt.bfloat16, kind="ExternalInput")
    if use_bias:
        bkv_d = nc.dram_tensor("bkv", [1, 256], dt.bfloat16, kind="ExternalInput")
        bq_d = nc.dram_tensor("bq", [1, 128], dt.bfloat16, kind="ExternalInput")
        bo_d = nc.dram_tensor("bo", [1, 128], dt.bfloat16, kind="ExternalInput")
        blep_d = nc.dram_tensor("blep", [1, 128], dt.bfloat16, kind="ExternalInput")
        bprj_d = nc.dram_tensor("bprj", [1, 256], dt.bfloat16, kind="ExternalInput")
    # output: channel-major partials, [oc-half, 128, n_tok]
    out_d = nc.dram_tensor("out", [2, 128, n_tok], dt.bfloat16,
                           kind="ExternalOutput")

    with tile.TileContext(nc) as tc:
        with (
            tc.tile_pool(name="const", bufs=1) as const,
            tc.tile_pool(name="work", bufs=2) as work,
            tc.tile_pool(name="psA", bufs=2, space="PSUM") as psA,
        ):
            # ---------------- constants / inputs
            xct = [const.tile([128, n_tok], dt.bfloat16, name=f"xct{k}")
                   for k in range(2)]
            for k in range(2):
                for q4 in range(4):   # split so chunk 0 can start early
                    qs_ = slice(q4 * (n_tok // 4), (q4 + 1) * (n_tok // 4))
                    nc.sync.dma_start(out=xct[k][:, qs_],
                                      in_=xct_d[128 * k:128 * (k + 1), qs_])

            def load(tname, dslice, shape, dtype=dt.bfloat16):
                t_ = const.tile(shape, dtype, name=tname)
                nc.sync.dma_start(out=t_, in_=dslice)
                return t_

            wkvtm = [load(f"wkvtm{k}", wkvtm_d[128 * k:128 * (k + 1), :],
                          [128, 256]) for k in range(2)]
            wq = [load(f"wq{k}", wq_d[128 * k:128 * (k + 1), :], [128, 128])
                  for k in range(2)]
            wo = [load(f"wo{k}", wo_d[128 * k:128 * (k + 1), :], [128, 128])
                  for k in range(2)]
            wp = load("wp", wp_d[:, :], [128, 256])
            dcw = load("dcw", dcw_d[:, :], [128, 384])
            rblk = load("rblk", rblk_d[:, :], [128, 128])
            pmat = load("pmat", pmat_d[:, :], [128, 128])
            hmask = load("hmask", hmask_d[:, :], [128, 128])
            id32 = load("id32", id32_d[:, :], [128, 128], dt.float32)
            ctm = load("ctm", ctm_d[:, :], [128, n_tok // 4])
            stm = load("stm", stm_d[:, :], [128, n_tok // 4])
            ccm = load("ccm", ccm_d[:, :], [128, n_tok])
            scm = load("scm", scm_d[:, :], [128, n_tok])
            ones = const.tile([128, CH], dt.bfloat16, name="ones")
            nc.vector.memset(ones, 1.0)
            id1 = const.tile([1, 1], dt.float32, name="id1")
            nc.vector.memset(id1, 1.0)
            if use_bias:
                bkv = load("bkv", bkv_d[:, :], [1, 256])
                bq = load("bq", bq_d[:, :], [1, 128])
                bo = load("bo", bo_d[:, :], [1, 128])
                blep = load("blep", blep_d[:, :], [1, 128])
                bprj = load("bprj", bprj_d[:, :], [1, 256])

            # persistent channel-major tensors
            vT = const.tile([128, th], dt.bfloat16, name="vT")
            nc.vector.memset(vT[:, 0:1], 0.0)
            nc.vector.memset(vT[:, th - 1:th], 0.0)
            o1p = const.tile([128, n_tok], dt.bfloat16, name="o1p")
            vpart = const.tile([128, nch], dt.float32, name="vpart")

            # stats PSUM: one bank per open accumulation group; the same
            # banks are recycled as phase-2 psum tiles via tag reuse
            gat = psA.tile([128, CH], dt.float32, tag="ga", name="gat")
            gbt = psA.tile([128, CH], dt.float32, tag="gb", bufs=1, name="gbt")
            krt = psA.tile([128, CH], dt.float32, tag="kr", bufs=1, name="krt")
            gram_a = gat[:, 0:128]
            gram_b = gbt[:, 0:128]
            krow = krt[0:1, 0:256]

            # =========================== phase A ===========================
            for c in range(nch):
                for hh in range(2):   # half chunk = 2 token tiles of 128
                    t0 = c * CH + hh * 256
                    first = (c == 0 and hh == 0)
                    last = (c == nch - 1 and hh == 1)
                    kv = psA.tile([128, 512], dt.float32, tag="kv", name="kv")
                    for l in range(2):
                        csl = slice(l * 256, (l + 1) * 256)
                        for k in range(2):
                            nc.tensor.matmul(
                                kv[:, csl],
                                xct[k][:, t0 + l * 128: t0 + (l + 1) * 128],
                                wkvtm[k],
                                start=(k == 0), stop=(k == 1 and not use_bias))
                        if use_bias:
                            nc.tensor.matmul(kv[:, csl], ones[0:1, 0:128], bkv,
                                             start=False, stop=True)
                    kv3 = kv.rearrange("p (l c) -> p l c", l=2)
                    kview = kv3[:, :, 0:128]
                    vview = kv3[:, :, 128:256]

                    et = work.tile([128, 256], dt.bfloat16, tag="et", name="et")
                    et3 = et.rearrange("p (l c) -> p l c", l=2)
                    nc.scalar.activation(et3, kview, AF.Exp)
                    rt = work.tile([128, 256], dt.bfloat16, tag="rt", name="rt")
                    rt3 = rt.rearrange("p (l c) -> p l c", l=2)
                    nc.vector.tensor_scalar_max(rt3, kview, 0.0)
                    k1 = work.tile([128, 256], dt.bfloat16, tag="k1", name="k1")
                    nc.vector.scalar_tensor_tensor(
                        out=k1, in0=et, scalar=1.0, in1=rt,
                        op0=OP.min, op1=OP.add)
                    vsb = work.tile([128, 256], dt.bfloat16, tag="vsb", name="vsb")
                    vsb3 = vsb.rearrange("p (l c) -> p l c", l=2)
                    nc.vector.tensor_copy(vsb3, vview)

                    # ksum row accumulation (k1 only)
                    nc.tensor.matmul(krow, ones[:, 0:1], k1,
                                     start=first, stop=last)

                    # rope factors: 4 heads share each 32-col sin/cos block
                    scol = (t0 // 128) * 32
                    cfac = ctm[:, scol:scol + 64] \
                        .rearrange("p (l c) -> p l c", l=2) \
                        .unsqueeze(2).to_broadcast((128, 2, 4, 32))
                    sfac = stm[:, scol:scol + 64] \
                        .rearrange("p (l c) -> p l c", l=2) \
                        .unsqueeze(2).to_broadcast((128, 2, 4, 32))
                    k1v = k1.rearrange("p (l h c) -> p l h c", l=2, h=4)
                    at = work.tile([128, 256], dt.bfloat16, tag="at", name="at")
                    nc.gpsimd.tensor_mul(
                        at.rearrange("p (l h c) -> p l h c", l=2, h=4), k1v, cfac)
                    wt = work.tile([128, 256], dt.bfloat16, tag="wt", name="wt")
                    nc.gpsimd.tensor_mul(
                        wt.rearrange("p (l h c) -> p l h c", l=2, h=4), k1v, sfac)

                    # gram accumulation
                    for l in range(2):
                        lsl = slice(l * 128, (l + 1) * 128)
                        nc.tensor.matmul(gram_a, at[:, lsl], vsb[:, lsl],
                                         start=(first and l == 0), stop=False)
                        nc.tensor.matmul(gram_b, wt[:, lsl], vsb[:, lsl],
                                         start=(first and l == 0),
                                         stop=(last and l == 1))

                # v channel-major for the conv + vsum
                vcm = psA.tile([128, CH], dt.float32, tag="vcm", name="vcm")
                for k in range(2):
                    nc.tensor.matmul(vcm, wkvtm[k][:, 128:256],
                                     xct[k][:, c * CH:(c + 1) * CH],
                                     start=(k == 0), stop=(k == 1 and not use_bias))
                if use_bias:
                    nc.tensor.matmul(vcm, bkv[:, 128:256], ones[0:1, :],
                                     start=False, stop=True)
                nc.scalar.activation(vT[:, 1 + c * CH: 1 + (c + 1) * CH], vcm,
                                     AF.Copy, accum_out=vpart[:, c:c + 1])

                # o channel-major for the phase-2 gate
                ocm = psA.tile([128, CH], dt.float32, tag="vcm", name="ocm")
                for k in range(2):
                    nc.tensor.matmul(ocm, wo[k], xct[k][:, c * CH:(c + 1) * CH],
                                     start=(k == 0), stop=(k == 1 and not use_bias))
                if use_bias:
                    nc.tensor.matmul(ocm, bo, ones[0:1, :],
                                     start=False, stop=True)
                nc.scalar.activation(o1p[:, c * CH:(c + 1) * CH], ocm, AF.Copy)

            # ======================= stats assembly ========================
            # gram = gram_a + P @ gram_b
            gbs = const.tile([128, 128], dt.bfloat16, name="gbs")
            nc.scalar.activation(gbs, gram_b, AF.Copy)
            nc.tensor.matmul(gram_a, pmat, gbs, start=False, stop=True)

            krs = const.tile([1, 256], dt.float32, name="krs")
            nc.vector.tensor_copy(krs, krow)
            ksumr = const.tile([1, 128], dt.float32, name="ksumr")
            nc.vector.tensor_add(ksumr, krs[:, 0:128], krs[:, 128:256])
            stps = psA.tile([128, CH], dt.float32, tag="vcm", name="stps")
            nc.tensor.transpose(stps[:, 0:1], ksumr, id1)
            zsc = const.tile([128, 1], dt.float32, name="zsc")
            nc.scalar.mul(zsc, stps[:, 0:1], SCALE / n_tok)
            vsumc = const.tile([128, 1], dt.float32, name="vsumc")
            nc.vector.tensor_reduce(vsumc, vpart, axis=mybir.AxisListType.X,
                                    op=OP.add)

            zblk = const.tile([128, 128], dt.bfloat16, name="zblk")
            nc.vector.tensor_tensor(zblk, zsc.to_broadcast((128, 128)), hmask,
                                    OP.mult)
            # mcorr[d, e] = -zsc[d] * vmean[e] * hmask[d, e]
            vcps = psA.tile([128, CH], dt.float32, tag="vcm", name="vcps")
            nc.tensor.transpose(vcps[0:1, 0:128], vsumc, id32)
            vrow = const.tile([1, 128], dt.float32, name="vrow")
            nc.scalar.mul(vrow, vcps[0:1, 0:128], -1.0 / n_tok)
            vrowb = const.tile([128, 128], dt.float32, name="vrowb")
            nc.gpsimd.partition_broadcast(vrowb, vrow)
            mc0 = const.tile([128, 128], dt.float32, name="mc0")
            nc.vector.tensor_tensor(mc0, zsc.to_broadcast((128, 128)), vrowb,
                                    OP.mult)
            mcorr = const.tile([128, 128], dt.bfloat16, name="mcorr")
            nc.vector.tensor_tensor(mcorr, mc0, hmask, OP.mult)

            kvb = const.tile([128, 128], dt.bfloat16, name="kvb")
            nc.vector.memset(kvb, 0.0)
            for a in range(4):
                psl = slice(32 * a, 32 * (a + 1))
                nc.scalar.mul(kvb[psl, psl], gram_a[psl, psl], SCALE / n_tok)

            # =========================== phase 2 ===========================
            for c in range(nch):
                tsl = slice(c * CH, (c + 1) * CH)
                xsl = [x[:, tsl] for x in xct]

                qps = psA.tile([128, CH], dt.float32, tag="kv", name="qps")
                for k in range(2):
                    nc.tensor.matmul(qps, wq[k], xsl[k], start=(k == 0),
                                     stop=(k == 1 and not use_bias))
                if use_bias:
                    nc.tensor.matmul(qps, bq, ones[0:1, :],
                                     start=False, stop=True)

                # elu(q)+1 = min(exp(q),1) + relu(q)
                eq = work.tile([128, CH], dt.bfloat16, tag="eq", name="eq")
                nc.scalar.activation(eq, qps, AF.Exp)
                rq = work.tile([128, CH], dt.bfloat16, tag="rq", name="rq")
                nc.scalar.activation(rq, qps, AF.Relu)
                q1 = work.tile([128, CH], dt.bfloat16, tag="q1", name="q1")
                nc.vector.scalar_tensor_tensor(
                    out=q1, in0=eq, scalar=1.0, in1=rq, op0=OP.min, op1=OP.add)

                # rope(q1)
                rot = psA.tile([128, CH], dt.float32, tag="ga", name="rot")
                nc.tensor.matmul(rot, rblk, q1, start=True, stop=True)
                m1 = work.tile([128, CH], dt.bfloat16, tag="m1", name="m1")
                nc.gpsimd.tensor_mul(m1, q1, ccm[:, tsl])
                m2 = work.tile([128, CH], dt.bfloat16, tag="m2", name="m2")
                nc.vector.tensor_mul(m2, rot, scm[:, tsl])
                qs = work.tile([128, CH], dt.bfloat16, tag="qs", name="qs")
                nc.gpsimd.tensor_add(qs, m1, m2)

                # z and 1 + 1/z
                zps = psA.tile([128, CH], dt.float32, tag="gb", bufs=1,
                               name="zps")
                nc.tensor.matmul(zps, zblk, q1, start=True, stop=True)
                rz = work.tile([128, CH], dt.float32, tag="rz", name="rz")
                nc.vector.reciprocal_approx_fast(out=rz, in_=zps)
                rp1 = work.tile([128, CH], dt.bfloat16, tag="rp1", name="rp1")
                nc.gpsimd.tensor_add(rp1, rz, ones)

                # attn = kvb^T qs ; rest = mcorr^T q1 + conv taps
                aps = psA.tile([128, CH], dt.float32, tag="kr", bufs=1,
                               name="aps")
                nc.tensor.matmul(aps, kvb, qs, start=True, stop=True)
                rst = psA.tile([128, CH], dt.float32, tag="kv", name="rst")
                nc.tensor.matmul(rst, mcorr, q1, start=True, stop=False)
                for tap in range(3):
                    nc.tensor.matmul(rst, dcw[:, tap * 128:(tap + 1) * 128],
                                     vT[:, c * CH + tap: c * CH + tap + CH],
                                     start=False, stop=(tap == 2 and not use_bias))
                if use_bias:
                    nc.tensor.matmul(rst, blep, ones[0:1, :],
                                     start=False, stop=True)

                t_ = work.tile([128, CH], dt.bfloat16, tag="t_", name="t_")
                nc.vector.tensor_mul(t_, aps, rp1)
                u_ = work.tile([128, CH], dt.bfloat16, tag="u_", name="u_")
                nc.vector.tensor_add(u_, rst, t_)
                y = work.tile([128, CH], dt.bfloat16, tag="y", name="y")
                nc.gpsimd.tensor_mul(y, u_, o1p[:, tsl])

                # partial output projection, channel-major halves
                outsb = work.tile([128, 2, CH], dt.bfloat16, tag="outsb",
                                  name="outsb")
                for j in range(2):
                    opj = psA.tile([128, CH], dt.float32, tag="vcm",
                                   name=f"opj{j}")
                    nc.tensor.matmul(opj, wp[:, j * 128:(j + 1) * 128], y,
                                     start=True, stop=(not use_bias))
                    if use_bias:
                        nc.tensor.matmul(opj, bprj[:, j * 128:(j + 1) * 128],
                                         ones[0:1, :], start=False, stop=True)
                    if j == 0:
                        nc.scalar.activation(outsb[:, j], opj, AF.Copy)
                    else:
                        nc.vector.tensor_copy(outsb[:, j], opj)
                nc.sync.dma_start(
                    out=out_d[:, :, tsl].rearrange("j p t -> p j t"), in_=outsb)

    nc.compile()
    return nc


_NC_CACHE = {}


def _get_nc(use_bias: bool):
    if use_bias not in _NC_CACHE:
        _NC_CACHE[use_bias] = _build_nc(use_bias)
    return _NC_CACHE[use_bias]


def kernel(x, sin, cos, W_qkvo, b_qkvo, W_lepe, b_lepe, W_proj, b_proj):
    from concourse.bass_utils import run_bass_kernel_spmd

    per_core, use_bias = _host_prep(x, sin, cos, W_qkvo, b_qkvo, W_lepe,
                                    b_lepe, W_proj, b_proj)
    nc = _get_nc(use_bias)
    import concourse.mybir as mybir
    expected = set()
    for alloc in nc.m.functions[0].allocations:
        if isinstance(alloc, mybir.MemoryLocationSet) and alloc.kind == "ExternalInput":
            expected.add(alloc.memorylocations[0].name)
    per_core = [{k: v for k, v in m.items() if k in expected} for m in per_core]
    res = run_bass_kernel_spmd(nc, per_core, core_ids=list(range(NCORES)),
                               trace=bool(os.environ.get("KERNEL_TRACE")))
    if os.environ.get("KERNEL_TRACE"):
        kernel.last_exec_time_ns = res.exec_time_ns
        kernel.last_results = res
    full = np.zeros((B, N, INTERNAL), np.float32)
    for b in range(B):
        s = (np.asarray(res.results[2 * b]["out"], np.float32)
             + np.asarray(res.results[2 * b + 1]["out"], np.float32))
        full[b] = s.reshape(256, N).T
    return full


# ---------------------------------------------------------- numpy reference
# numpy emulation of the device decomposition (fp32), for fast validation.

def _numpy_pipeline(per_core_inputs):
    outs = []
    for c in range(NCORES):
        d = per_core_inputs[c]
        xf = d["xct"].astype(np.float32)            # [256, n]
        wkvtm = d["wkvtm"].astype(np.float32)       # [256, 256]
        wq = d["wq"].astype(np.float32)
        wo = d["wo"].astype(np.float32)
        wp = d["wp"].astype(np.float32)
        dcw = d["dcw"].astype(np.float32).reshape(128, 3, 128)
        R = d["rblk"].astype(np.float32)
        P = d["pmat"].astype(np.float32)
        hmask = d["hmask"].astype(np.float32)
        ctm = d["ctm"].astype(np.float32)
        stm = d["stm"].astype(np.float32)
        ccm = d["ccm"].astype(np.float32)
        scm = d["scm"].astype(np.float32)
        n_tok = xf.shape[1]

        def elu1(t):
            return np.minimum(np.exp(t), 1.0) + np.maximum(t, 0.0)

        # ---- phase A (token-major k/v)
        kv_tm = xf.T @ wkvtm                        # [n, 256] = [k|v]
        k_tm = elu1(kv_tm[:, 0:128])
        v_tm = kv_tm[:, 128:256]
        ntile = n_tok // 128
        cos_tm = ctm.reshape(128, ntile, 32).transpose(1, 0, 2).reshape(n_tok, 32)
        ssw_tm = stm.reshape(128, ntile, 32).transpose(1, 0, 2).reshape(n_tok, 32)
        a_tm = k_tm * np.tile(cos_tm, (1, 4))
        w_tm = k_tm * np.tile(ssw_tm, (1, 4))
        gram = a_tm.T @ v_tm + P @ (w_tm.T @ v_tm)  # [128, 128]
        ksum = k_tm.sum(axis=0)
        v_cm = wkvtm[:, 128:256].T @ xf             # [128, n]
        vsum = v_cm.sum(axis=1)
        vTf = np.concatenate([np.zeros((128, 1), np.float32), v_cm,
                              np.zeros((128, 1), np.float32)], axis=1)
        o_cm = wo.T @ xf

        zsc = SCALE / n_tok * ksum
        zblk = zsc[:, None] * hmask
        kvblk = np.zeros((128, 128), np.float32)
        for a in range(4):
            s2 = slice(32 * a, 32 * (a + 1))
            kvblk[s2, s2] = (SCALE / n_tok) * gram[s2, s2]
        mcorr = -zsc[:, None] * (vsum[None, :] / n_tok) * hmask

        # ---- phase 2
        q1 = elu1(wq.T @ xf)
        qsh = q1 * ccm[:, :n_tok] + (R.T @ q1) * scm[:, :n_tok]
        z = zblk.T @ q1
        rp1 = 1.0 + 1.0 / z
        attn = kvblk.T @ qsh
        rest = mcorr.T @ q1
        for tap in range(3):
            rest += dcw[:, tap, :].T @ vTf[:, tap:tap + n_tok]
        y = (attn * rp1 + rest) * o_cm
        outs.append(np.stack([wp[:, 0:128].T @ y, wp[:, 128:256].T @ y]))

    n_tok = outs[0].shape[2]
    full = np.zeros((B, n_tok, 256), np.float32)
    for b in range(B):
        s = outs[2 * b] + outs[2 * b + 1]
        full[b] = s.reshape(256, n_tok).T
    return full


if __name__ == "__main__" and os.environ.get("KERNEL_SELFTEST"):
    sys.path.insert(0, os.path.dirname(os.path.abspath(__file__)))
    os.environ.setdefault("JAX_PLATFORMS", "cpu")
    import reference
    inputs = {k: np.asarray(v) for k, v in reference.setup_inputs().items()}
    expected = np.asarray(reference.reference(**inputs))
    per_core, use_bias = _host_prep(**inputs)
    got = _numpy_pipeline(per_core)
    err = np.abs(got - expected)
    rel = np.linalg.norm(got - expected) / np.linalg.norm(expected)
    print("selftest rel err:", rel, "max abs:", err.max())

if __name__ == "__main__" and os.environ.get("KERNEL_BUILD"):
    nc = _build_nc(False)
    import tempfile
    from concourse.bass_utils import compile_bass_kernel
    print("NEFF:", compile_bass_kernel(nc, tempfile.mkdtemp()))


# revision 10
# speedup vs baseline: 1.6353x; 1.2362x over previous
"""Trainium2 Bass kernel for nn_MultiHeadMALAAttention.

Sharding (tensor-parallel over heads): 8 cores; core c handles batch
b = c//2 and head-group hg = c%2 (4 heads = 128 internal channels) over
ALL N=8192 tokens.  kv_state / ksum / vsum / z are per-head -> fully
core-local, no collective.  The output projection is a partial sum over
the core's 128 channels; the host adds the two partials of each batch.

Device pipeline per core:
  Phase A (per 512-token chunk): k,v projected TOKEN-major directly
    (lhsT = x channel-major slice, rhs = W) -> elu(k)+1 via
    min(exp,1)+relu -> rope folded into the kv gram: accumulate
    gram_a += (k1*cos)^T v and gram_b += (k1*sswap)^T v, where sswap is
    the pair-swapped+signed sin (host precomputed); after the loop
    gram = gram_a + P @ gram_b with P the pair-swap permutation (one
    matmul).  v and o are also produced channel-major (v for the LEPE
    conv + vsum, o for the phase-2 gate).  ksum via ones-matmul on k1.
  Phase 2 (per 512-token chunk): q projected channel-major; elu; rope
    of q via the R block matmul; z = zblk^T q1; attn = kvb^T qs;
    rest = mcorr^T q1 + depthwise conv taps (diag matmuls);
    y = (attn*(1+1/z) + rest) * o; out = wp^T y (channel-major bf16
    partials) -> DMA.  Host: transpose + add the two partials per batch.
"""

import os
import sys

sys.path.insert(0, "/opt/trn_rl_repo")

import numpy as np
import ml_dtypes

B, N, DIM, H, HD = 4, 8192, 256, 8, 32
INTERNAL = H * HD  # 256
SCALE = HD ** -0.5
NCORES = 8

CH = 512            # chunk tokens
NCH = N // CH       # 16 chunks per core
TH = N + 2          # vT with 1-token zero halo each side

BF16 = ml_dtypes.bfloat16


# ---------------------------------------------------------------- host prep

def _host_prep(x, sin, cos, W_qkvo, b_qkvo, W_lepe, b_lepe, W_proj, b_proj):
    """Build per-core input dicts (all device tensors)."""
    WT = np.asarray(W_qkvo, np.float32).T          # [DIM, 1024] lhsT layout
    WPT = np.asarray(W_proj, np.float32).T         # [INTERNAL, DIM]
    wl = np.asarray(W_lepe, np.float32)[:, 0, :]   # [256, 3]
    sinf = np.asarray(sin, np.float32)             # [N, 32]
    cosf = np.asarray(cos, np.float32)
    xf = np.asarray(x, np.float32)

    # R: rot = R.T @ x ; rot[2i] = -x[2i+1], rot[2i+1] = x[2i]
    R = np.zeros((128, 128), np.float32)
    for i in range(64):
        R[2 * i + 1, 2 * i] = -1.0
        R[2 * i, 2 * i + 1] = 1.0
    rblk = R.astype(BF16)

    # P: pair-swap permutation (symmetric)
    P = np.zeros((128, 128), np.float32)
    for i in range(64):
        P[2 * i, 2 * i + 1] = 1.0
        P[2 * i + 1, 2 * i] = 1.0
    pmat = P.astype(BF16)

    hmask = np.zeros((128, 128), np.float32)
    for hh in range(4):
        hmask[32 * hh:32 * (hh + 1), 32 * hh:32 * (hh + 1)] = 1.0
    hmask = hmask.astype(BF16)

    id32 = np.eye(128, dtype=np.float32)

    # token-major compact sin/cos for the k rope: [128, 32] per 128-token
    # tile -> [128, N/4].  stm is the swapped+signed sin:
    # sswap[t, 2i] = sin[t, 2i+1], sswap[t, 2i+1] = -sin[t, 2i]
    sswap = np.empty_like(sinf)
    sswap[:, 0::2] = sinf[:, 1::2]
    sswap[:, 1::2] = -sinf[:, 0::2]
    ntile = N // 128
    ctm = np.ascontiguousarray(
        cosf.reshape(ntile, 128, 32).transpose(1, 0, 2).reshape(128, N // 4)
    ).astype(BF16)
    stm = np.ascontiguousarray(
        sswap.reshape(ntile, 128, 32).transpose(1, 0, 2).reshape(128, N // 4)
    ).astype(BF16)

    # channel-major sin/cos for the q rope: [128, N], rows = 4 heads x 32
    ccm = np.ascontiguousarray(np.tile(cosf.T, (4, 1))).astype(BF16)
    scm = np.ascontiguousarray(np.tile(sinf.T, (4, 1))).astype(BF16)

    use_bias = bool(np.any(b_qkvo) or np.any(b_lepe) or np.any(b_proj))

    shared = {"rblk": rblk, "pmat": pmat, "hmask": hmask, "id32": id32,
              "ctm": ctm, "stm": stm, "ccm": ccm, "scm": scm}
    per_core = []
    xcts = {}
    for c in range(NCORES):
        b = c // 2
        hg = c % 2
        osl = slice(hg * 128, hg * 128 + 128)
        if b not in xcts:
            xcts[b] = np.ascontiguousarray(xf[b].T).astype(BF16)  # [256, N]
        # [dim, k-chans own | v-chans own]
        wkvtm = np.ascontiguousarray(np.concatenate(
            [WT[:, 256 + hg * 128:256 + hg * 128 + 128],
             WT[:, 512 + hg * 128:512 + hg * 128 + 128]], axis=1)).astype(BF16)
        wq = np.ascontiguousarray(WT[:, hg * 128:hg * 128 + 128]).astype(BF16)
        wo = np.ascontiguousarray(
            WT[:, 768 + hg * 128:768 + hg * 128 + 128]).astype(BF16)
        wp = np.ascontiguousarray(WPT[osl, :]).astype(BF16)       # [128, 256]

        wlo = wl[osl]                                             # [128, 3]
        dcw = np.zeros((128, 3, 128), np.float32)
        for tap in range(3):
            np.fill_diagonal(dcw[:, tap, :], wlo[:, tap])
        dcw = dcw.reshape(128, 384).astype(BF16)

        d = {"xct": xcts[b], "wkvtm": wkvtm, "wq": wq, "wo": wo, "wp": wp,
             "dcw": dcw}
        d.update(shared)
        if use_bias:
            bq = np.asarray(b_qkvo, np.float32)
            d["bkv"] = np.ascontiguousarray(np.concatenate(
                [bq[256 + hg * 128:256 + hg * 128 + 128],
                 bq[512 + hg * 128:512 + hg * 128 + 128]]
            ).reshape(1, 256)).astype(BF16)
            d["bq"] = np.ascontiguousarray(
                bq[hg * 128:hg * 128 + 128].reshape(1, 128)).astype(BF16)
            d["bo"] = np.ascontiguousarray(
                bq[768 + hg * 128:768 + hg * 128 + 128].reshape(1, 128)
            ).astype(BF16)
            d["blep"] = np.ascontiguousarray(
                np.asarray(b_lepe, np.float32)[osl].reshape(1, 128)).astype(BF16)
            d["bprj"] = np.ascontiguousarray(
                np.asarray(b_proj, np.float32).reshape(1, 256)).astype(BF16)
        per_core.append(d)
    return per_core, use_bias


# ------------------------------------------------------------ device kernel

def _build_nc(use_bias: bool, nch: int = NCH):
    from concourse import bacc
    import concourse.mybir as mybir
    import concourse.tile as tile

    dt = mybir.dt
    AF = mybir.ActivationFunctionType
    OP = mybir.AluOpType

    n_tok = nch * CH
    th = n_tok + 2

    nc = bacc.Bacc(None, target_bir_lowering=False)

    xct_d = nc.dram_tensor("xct", [256, n_tok], dt.bfloat16, kind="ExternalInput")
    wkvtm_d = nc.dram_tensor("wkvtm", [256, 256], dt.bfloat16, kind="ExternalInput")
    wq_d = nc.dram_tensor("wq", [256, 128], dt.bfloat16, kind="ExternalInput")
    wo_d = nc.dram_tensor("wo", [256, 128], dt.bfloat16, kind="ExternalInput")
    wp_d = nc.dram_tensor("wp", [128, 256], dt.bfloat16, kind="ExternalInput")
    dcw_d = nc.dram_tensor("dcw", [128, 384], dt.bfloat16, kind="ExternalInput")
    rblk_d = nc.dram_tensor("rblk", [128, 128], dt.bfloat16, kind="ExternalInput")
    pmat_d = nc.dram_tensor("pmat", [128, 128], dt.bfloat16, kind="ExternalInput")
    hmask_d = nc.dram_tensor("hmask", [128, 128], dt.bfloat16, kind="ExternalInput")
    id32_d = nc.dram_tensor("id32", [128, 128], dt.float32, kind="ExternalInput")
    ctm_d = nc.dram_tensor("ctm", [128, n_tok // 4], dt.bfloat16,
                           kind="ExternalInput")
    stm_d = nc.dram_tensor("stm", [128, n_tok // 4], dt.bfloat16,
                           kind="ExternalInput")
    ccm_d = nc.dram_tensor("ccm", [128, n_tok], dt.bfloat16, kind="ExternalInput")
    scm_d = nc.dram_tensor("scm", [128, n_tok], dt.bfloat16, kind="ExternalInput")
    if use_bias:
        bkv_d = nc.dram_tensor("bkv", [1, 256], dt.bfloat16, kind="ExternalInput")
        bq_d = nc.dram_tensor("bq", [1, 128], dt.bfloat16, kind="ExternalInput")
        bo_d = nc.dram_tensor("bo", [1, 128], dt.bfloat16, kind="ExternalInput")
        blep_d = nc.dram_tensor("blep", [1, 128], dt.bfloat16, kind="ExternalInput")
        bprj_d = nc.dram_tensor("bprj", [1, 256], dt.bfloat16, kind="ExternalInput")
    # output: channel-major partials, [oc-half, 128, n_tok]
    out_d = nc.dram_tensor("out", [2, 128, n_tok], dt.bfloat16,
                           kind="ExternalOutput")

    with tile.TileContext(nc) as tc:
        with (
            tc.tile_pool(name="const", bufs=1) as const,
            tc.tile_pool(name="work", bufs=2) as work,
            tc.tile_pool(name="psA", bufs=2, space="PSUM") as psA,
        ):
            # ---------------- constants / inputs
            xct = [const.tile([128, n_tok], dt.bfloat16, name=f"xct{k}")
                   for k in range(2)]
            for k in range(2):
                for q4 in range(4):   # split so chunk 0 can start early
                    qs_ = slice(q4 * (n_tok // 4), (q4 + 1) * (n_tok // 4))
                    nc.sync.dma_start(out=xct[k][:, qs_],
                                      in_=xct_d[128 * k:128 * (k + 1), qs_])

            def load(tname, dslice, shape, dtype=dt.bfloat16):
                t_ = const.tile(shape, dtype, name=tname)
                nc.sync.dma_start(out=t_, in_=dslice)
                return t_

            wkvtm = [load(f"wkvtm{k}", wkvtm_d[128 * k:128 * (k + 1), :],
                          [128, 256]) for k in range(2)]
            wq = [load(f"wq{k}", wq_d[128 * k:128 * (k + 1), :], [128, 128])
                  for k in range(2)]
            wo = [load(f"wo{k}", wo_d[128 * k:128 * (k + 1), :], [128, 128])
                  for k in range(2)]
            wp = load("wp", wp_d[:, :], [128, 256])
            dcw = load("dcw", dcw_d[:, :], [128, 384])
            rblk = load("rblk", rblk_d[:, :], [128, 128])
            pmat = load("pmat", pmat_d[:, :], [128, 128])
            hmask = load("hmask", hmask_d[:, :], [128, 128])
            id32 = load("id32", id32_d[:, :], [128, 128], dt.float32)
            ctm = load("ctm", ctm_d[:, :], [128, n_tok // 4])
            stm = load("stm", stm_d[:, :], [128, n_tok // 4])
            ccm = load("ccm", ccm_d[:, :], [128, n_tok])
            scm = load("scm", scm_d[:, :], [128, n_tok])
            ones = const.tile([128, CH], dt.bfloat16, name="ones")
            nc.vector.memset(ones, 1.0)
            id1 = const.tile([1, 1], dt.float32, name="id1")
            nc.vector.memset(id1, 1.0)
            if use_bias:
                bkv = load("bkv", bkv_d[:, :], [1, 256])
                bq = load("bq", bq_d[:, :], [1, 128])
                bo = load("bo", bo_d[:, :], [1, 128])
                blep = load("blep", blep_d[:, :], [1, 128])
                bprj = load("bprj", bprj_d[:, :], [1, 256])

            # persistent channel-major tensors
            vT = const.tile([128, th], dt.bfloat16, name="vT")
            nc.vector.memset(vT[:, 0:1], 0.0)
            nc.vector.memset(vT[:, th - 1:th], 0.0)
            o1p = const.tile([128, n_tok], dt.bfloat16, name="o1p")
            vpart = const.tile([128, nch], dt.float32, name="vpart")

            # stats PSUM: one bank per open accumulation group; the same
            # banks are recycled as phase-2 psum tiles via tag reuse
            gat = psA.tile([128, CH], dt.float32, tag="ga", name="gat")
            gbt = psA.tile([128, CH], dt.float32, tag="gb", bufs=1, name="gbt")
            krt = psA.tile([128, CH], dt.float32, tag="kr", bufs=1, name="krt")
            gram_a = gat[:, 0:128]
            gram_b = gbt[:, 0:128]
            krow = krt[0:1, 0:256]

            # =========================== phase A ===========================
            # Software-pipelined: the gram/ksum matmuls of each half-chunk
            # are emitted after the NEXT half's kv projection so the PE
            # streams while ACT/DVE/GpSimd produce the rope factors.
            halves = [(c, hh) for c in range(nch) for hh in range(2)]

            def emit_gram(s):
                for l in range(2):
                    lsl = slice(l * 128, (l + 1) * 128)
                    nc.tensor.matmul(gram_a, s["at"][:, lsl], s["vsb"][:, lsl],
                                     start=(s["first"] and l == 0), stop=False)
                    nc.tensor.matmul(gram_b, s["wt"][:, lsl], s["vsb"][:, lsl],
                                     start=(s["first"] and l == 0),
                                     stop=(s["last"] and l == 1))
                nc.tensor.matmul(krow, ones[:, 0:1], s["k1"],
                                 start=s["first"], stop=s["last"])

            prev = None
            for idx, (c, hh) in enumerate(halves):
                t0 = c * CH + hh * 256
                kv = psA.tile([128, 512], dt.float32, tag="kv", name="kv")
                for l in range(2):
                    csl = slice(l * 256, (l + 1) * 256)
                    for k in range(2):
                        nc.tensor.matmul(
                            kv[:, csl],
                            xct[k][:, t0 + l * 128: t0 + (l + 1) * 128],
                            wkvtm[k],
                            start=(k == 0), stop=(k == 1 and not use_bias))
                    if use_bias:
                        nc.tensor.matmul(kv[:, csl], ones[0:1, 0:128], bkv,
                                         start=False, stop=True)
                if prev is not None:
                    emit_gram(prev)
                if hh == 1:
                    # v channel-major for the conv + vsum
                    vcm = psA.tile([128, CH], dt.float32, tag="vcm", name="vcm")
                    for k in range(2):
                        nc.tensor.matmul(vcm, wkvtm[k][:, 128:256],
                                         xct[k][:, c * CH:(c + 1) * CH],
                                         start=(k == 0),
                                         stop=(k == 1 and not use_bias))
                    if use_bias:
                        nc.tensor.matmul(vcm, bkv[:, 128:256], ones[0:1, :],
                                         start=False, stop=True)
                    nc.scalar.activation(vT[:, 1 + c * CH: 1 + (c + 1) * CH],
                                         vcm, AF.Copy,
                                         accum_out=vpart[:, c:c + 1])
                    # o channel-major for the phase-2 gate
                    ocm = psA.tile([128, CH], dt.float32, tag="vcm", name="ocm")
                    for k in range(2):
                        nc.tensor.matmul(ocm, wo[k],
                                         xct[k][:, c * CH:(c + 1) * CH],
                                         start=(k == 0),
                                         stop=(k == 1 and not use_bias))
                    if use_bias:
                        nc.tensor.matmul(ocm, bo, ones[0:1, :],
                                         start=False, stop=True)
                    nc.scalar.activation(o1p[:, c * CH:(c + 1) * CH], ocm,
                                         AF.Copy)

                kv3 = kv.rearrange("p (l c) -> p l c", l=2)
                kview = kv3[:, :, 0:128]
                vview = kv3[:, :, 128:256]
                et = work.tile([128, 256], dt.bfloat16, tag="et", name="et")
                nc.scalar.activation(et.rearrange("p (l c) -> p l c", l=2),
                                     kview, AF.Exp)
                rt = work.tile([128, 256], dt.bfloat16, tag="rt", name="rt")
                nc.vector.tensor_scalar_max(
                    rt.rearrange("p (l c) -> p l c", l=2), kview, 0.0)
                k1 = work.tile([128, 256], dt.bfloat16, tag="k1", bufs=3,
                               name="k1")
                nc.vector.scalar_tensor_tensor(
                    out=k1, in0=et, scalar=1.0, in1=rt, op0=OP.min, op1=OP.add)
                vsb = work.tile([128, 256], dt.bfloat16, tag="vsb", name="vsb")
                vsb3 = vsb.rearrange("p (l c) -> p l c", l=2)
                if hh == 0:
                    nc.scalar.activation(vsb3, vview, AF.Copy)
                else:
                    nc.vector.tensor_copy(vsb3, vview)

                scol = (t0 // 128) * 32
                cfac = ctm[:, scol:scol + 64] \
                    .rearrange("p (l c) -> p l c", l=2) \
                    .unsqueeze(2).to_broadcast((128, 2, 4, 32))
                sfac = stm[:, scol:scol + 64] \
                    .rearrange("p (l c) -> p l c", l=2) \
                    .unsqueeze(2).to_broadcast((128, 2, 4, 32))
                k1v = k1.rearrange("p (l h c) -> p l h c", l=2, h=4)
                at = work.tile([128, 256], dt.bfloat16, tag="at", name="at")
                nc.gpsimd.tensor_mul(
                    at.rearrange("p (l h c) -> p l h c", l=2, h=4), k1v, cfac)
                wt = work.tile([128, 256], dt.bfloat16, tag="wt", name="wt")
                nc.gpsimd.tensor_mul(
                    wt.rearrange("p (l h c) -> p l h c", l=2, h=4), k1v, sfac)

                prev = {"at": at, "wt": wt, "vsb": vsb, "k1": k1,
                        "first": idx == 0, "last": idx == len(halves) - 1}
            emit_gram(prev)

            # ======================= stats assembly ========================
            # gram = gram_a + P @ gram_b
            gbs = const.tile([128, 128], dt.bfloat16, name="gbs")
            nc.scalar.activation(gbs, gram_b, AF.Copy)
            nc.tensor.matmul(gram_a, pmat, gbs, start=False, stop=True)

            krs = const.tile([1, 256], dt.float32, name="krs")
            nc.vector.tensor_copy(krs, krow)
            ksumr = const.tile([1, 128], dt.float32, name="ksumr")
            nc.vector.tensor_add(ksumr, krs[:, 0:128], krs[:, 128:256])
            stps = psA.tile([128, CH], dt.float32, tag="vcm", name="stps")
            nc.tensor.transpose(stps[:, 0:1], ksumr, id1)
            zsc = const.tile([128, 1], dt.float32, name="zsc")
            nc.scalar.mul(zsc, stps[:, 0:1], SCALE / n_tok)
            vsumc = const.tile([128, 1], dt.float32, name="vsumc")
            nc.vector.tensor_reduce(vsumc, vpart, axis=mybir.AxisListType.X,
                                    op=OP.add)

            zblk = const.tile([128, 128], dt.bfloat16, name="zblk")
            nc.vector.tensor_tensor(zblk, zsc.to_broadcast((128, 128)), hmask,
                                    OP.mult)
            # mcorr[d, e] = -zsc[d] * vmean[e] * hmask[d, e]
            vcps = psA.tile([128, CH], dt.float32, tag="vcm", name="vcps")
            nc.tensor.transpose(vcps[0:1, 0:128], vsumc, id32)
            vrow = const.tile([1, 128], dt.float32, name="vrow")
            nc.scalar.mul(vrow, vcps[0:1, 0:128], -1.0 / n_tok)
            vrowb = const.tile([128, 128], dt.float32, name="vrowb")
            nc.gpsimd.partition_broadcast(vrowb, vrow)
            mc0 = const.tile([128, 128], dt.float32, name="mc0")
            nc.vector.tensor_tensor(mc0, zsc.to_broadcast((128, 128)), vrowb,
                                    OP.mult)
            mcorr = const.tile([128, 128], dt.bfloat16, name="mcorr")
            nc.vector.tensor_tensor(mcorr, mc0, hmask, OP.mult)

            kvb = const.tile([128, 128], dt.bfloat16, name="kvb")
            nc.vector.memset(kvb, 0.0)
            for a in range(4):
                psl = slice(32 * a, 32 * (a + 1))
                nc.scalar.mul(kvb[psl, psl], gram_a[psl, psl], SCALE / n_tok)

            # =========================== phase 2 ===========================
            # 4-stage software pipeline: S0 proj+elu, S1 rope/z, S2 attn/rest,
            # S3 output projection + DMA.  Stage s of chunk c is emitted at
            # iteration c+s so each stage's matmuls overlap the previous
            # stages' elementwise work on ACT/DVE/GpSimd.
            def S0(c):
                tsl = slice(c * CH, (c + 1) * CH)
                qps = psA.tile([128, CH], dt.float32, tag="kv", name="qps")
                for k in range(2):
                    nc.tensor.matmul(qps, wq[k], xct[k][:, tsl],
                                     start=(k == 0),
                                     stop=(k == 1 and not use_bias))
                if use_bias:
                    nc.tensor.matmul(qps, bq, ones[0:1, :],
                                     start=False, stop=True)
                eq = work.tile([128, CH], dt.bfloat16, tag="eq", name="eq")
                nc.scalar.activation(eq, qps, AF.Exp)
                rq = work.tile([128, CH], dt.bfloat16, tag="rq", name="rq")
                nc.scalar.activation(rq, qps, AF.Relu)
                q1 = work.tile([128, CH], dt.bfloat16, tag="q1", bufs=4,
                               name="q1")
                nc.vector.scalar_tensor_tensor(
                    out=q1, in0=eq, scalar=1.0, in1=rq, op0=OP.min, op1=OP.add)
                return {"c": c, "q1": q1}

            def S1(s):
                c = s["c"]
                tsl = slice(c * CH, (c + 1) * CH)
                rot = psA.tile([128, CH], dt.float32, tag="ga", name="rot")
                nc.tensor.matmul(rot, rblk, s["q1"], start=True, stop=True)
                zps = psA.tile([128, CH], dt.float32, tag="gb", bufs=1,
                               name="zps")
                nc.tensor.matmul(zps, zblk, s["q1"], start=True, stop=True)
                m1 = work.tile([128, CH], dt.bfloat16, tag="m1", bufs=3,
                               name="m1")
                nc.gpsimd.tensor_mul(m1, s["q1"], ccm[:, tsl])
                m2 = work.tile([128, CH], dt.bfloat16, tag="m2", bufs=3,
                               name="m2")
                nc.vector.tensor_mul(m2, rot, scm[:, tsl])
                rz = work.tile([128, CH], dt.float32, tag="rz", name="rz")
                nc.vector.reciprocal_approx_fast(out=rz, in_=zps)
                rp1 = work.tile([128, CH], dt.bfloat16, tag="rp1", bufs=3,
                                name="rp1")
                nc.scalar.activation(rp1, rz, AF.Copy, bias=1.0)
                s.update(m1=m1, m2=m2, rp1=rp1)

            def S2(s):
                c = s["c"]
                aps = psA.tile([128, CH], dt.float32, tag="kr", bufs=1,
                               name="aps")
                nc.tensor.matmul(aps, kvb, s["m1"], start=True, stop=False)
                nc.tensor.matmul(aps, kvb, s["m2"], start=False, stop=True)
                rst = psA.tile([128, CH], dt.float32, tag="ga", name="rst")
                nc.tensor.matmul(rst, mcorr, s["q1"], start=True, stop=False)
                for tap in range(3):
                    nc.tensor.matmul(rst, dcw[:, tap * 128:(tap + 1) * 128],
                                     vT[:, c * CH + tap: c * CH + tap + CH],
                                     start=False,
                                     stop=(tap == 2 and not use_bias))
                if use_bias:
                    nc.tensor.matmul(rst, blep, ones[0:1, :],
                                     start=False, stop=True)
                t_ = work.tile([128, CH], dt.bfloat16, tag="t_", name="t_")
                nc.vector.tensor_mul(t_, aps, s["rp1"])
                u_ = work.tile([128, CH], dt.bfloat16, tag="u_", name="u_")
                nc.vector.tensor_add(u_, rst, t_)
                y = work.tile([128, CH], dt.bfloat16, tag="y", bufs=3, name="y")
                nc.gpsimd.tensor_mul(y, u_, o1p[:, c * CH:(c + 1) * CH])
                s.update(y=y)

            def S3(s):
                c = s["c"]
                tsl = slice(c * CH, (c + 1) * CH)
                outsb = work.tile([128, 2, CH], dt.bfloat16, tag="outsb",
                                  name="outsb")
                for j in range(2):
                    opj = psA.tile([128, CH], dt.float32, tag="vcm",
                                   name=f"opj{j}")
                    nc.tensor.matmul(opj, wp[:, j * 128:(j + 1) * 128],
                                     s["y"], start=True, stop=(not use_bias))
                    if use_bias:
                        nc.tensor.matmul(opj, bprj[:, j * 128:(j + 1) * 128],
                                         ones[0:1, :], start=False, stop=True)
                    if j == 0:
                        nc.scalar.activation(outsb[:, j], opj, AF.Copy)
                    else:
                        nc.vector.tensor_copy(outsb[:, j], opj)
                nc.sync.dma_start(
                    out=out_d[:, :, tsl].rearrange("j p t -> p j t"), in_=outsb)

            pipe = [None, None, None]
            for i in range(nch + 3):
                if i < nch:
                    s0 = S0(i)
                else:
                    s0 = None
                if pipe[0] is not None:
                    S1(pipe[0])
                if pipe[1] is not None:
                    S2(pipe[1])
                if pipe[2] is not None:
                    S3(pipe[2])
                pipe = [s0, pipe[0], pipe[1]]

    nc.compile()
    return nc


_NC_CACHE = {}


def _get_nc(use_bias: bool):
    if use_bias not in _NC_CACHE:
        _NC_CACHE[use_bias] = _build_nc(use_bias)
    return _NC_CACHE[use_bias]


def kernel(x, sin, cos, W_qkvo, b_qkvo, W_lepe, b_lepe, W_proj, b_proj):
    from concourse.bass_utils import run_bass_kernel_spmd

    per_core, use_bias = _host_prep(x, sin, cos, W_qkvo, b_qkvo, W_lepe,
                                    b_lepe, W_proj, b_proj)
    nc = _get_nc(use_bias)
    import concourse.mybir as mybir
    expected = set()
    for alloc in nc.m.functions[0].allocations:
        if isinstance(alloc, mybir.MemoryLocationSet) and alloc.kind == "ExternalInput":
            expected.add(alloc.memorylocations[0].name)
    per_core = [{k: v for k, v in m.items() if k in expected} for m in per_core]
    res = run_bass_kernel_spmd(nc, per_core, core_ids=list(range(NCORES)),
                               trace=bool(os.environ.get("KERNEL_TRACE")))
    if os.environ.get("KERNEL_TRACE"):
        kernel.last_exec_time_ns = res.exec_time_ns
        kernel.last_results = res
    full = np.zeros((B, N, INTERNAL), np.float32)
    for b in range(B):
        s = (np.asarray(res.results[2 * b]["out"], np.float32)
             + np.asarray(res.results[2 * b + 1]["out"], np.float32))
        full[b] = s.reshape(256, N).T
    return full


# ---------------------------------------------------------- numpy reference
# numpy emulation of the device decomposition (fp32), for fast validation.

def _numpy_pipeline(per_core_inputs):
    outs = []
    for c in range(NCORES):
        d = per_core_inputs[c]
        xf = d["xct"].astype(np.float32)            # [256, n]
        wkvtm = d["wkvtm"].astype(np.float32)       # [256, 256]
        wq = d["wq"].astype(np.float32)
        wo = d["wo"].astype(np.float32)
        wp = d["wp"].astype(np.float32)
        dcw = d["dcw"].astype(np.float32).reshape(128, 3, 128)
        R = d["rblk"].astype(np.float32)
        P = d["pmat"].astype(np.float32)
        hmask = d["hmask"].astype(np.float32)
        ctm = d["ctm"].astype(np.float32)
        stm = d["stm"].astype(np.float32)
        ccm = d["ccm"].astype(np.float32)
        scm = d["scm"].astype(np.float32)
        n_tok = xf.shape[1]

        def elu1(t):
            return np.minimum(np.exp(t), 1.0) + np.maximum(t, 0.0)

        # ---- phase A (token-major k/v)
        kv_tm = xf.T @ wkvtm                        # [n, 256] = [k|v]
        k_tm = elu1(kv_tm[:, 0:128])
        v_tm = kv_tm[:, 128:256]
        ntile = n_tok // 128
        cos_tm = ctm.reshape(128, ntile, 32).transpose(1, 0, 2).reshape(n_tok, 32)
        ssw_tm = stm.reshape(128, ntile, 32).transpose(1, 0, 2).reshape(n_tok, 32)
        a_tm = k_tm * np.tile(cos_tm, (1, 4))
        w_tm = k_tm * np.tile(ssw_tm, (1, 4))
        gram = a_tm.T @ v_tm + P @ (w_tm.T @ v_tm)  # [128, 128]
        ksum = k_tm.sum(axis=0)
        v_cm = wkvtm[:, 128:256].T @ xf             # [128, n]
        vsum = v_cm.sum(axis=1)
        vTf = np.concatenate([np.zeros((128, 1), np.float32), v_cm,
                              np.zeros((128, 1), np.float32)], axis=1)
        o_cm = wo.T @ xf

        zsc = SCALE / n_tok * ksum
        zblk = zsc[:, None] * hmask
        kvblk = np.zeros((128, 128), np.float32)
        for a in range(4):
            s2 = slice(32 * a, 32 * (a + 1))
            kvblk[s2, s2] = (SCALE / n_tok) * gram[s2, s2]
        mcorr = -zsc[:, None] * (vsum[None, :] / n_tok) * hmask

        # ---- phase 2
        q1 = elu1(wq.T @ xf)
        qsh = q1 * ccm[:, :n_tok] + (R.T @ q1) * scm[:, :n_tok]
        z = zblk.T @ q1
        rp1 = 1.0 + 1.0 / z
        attn = kvblk.T @ qsh
        rest = mcorr.T @ q1
        for tap in range(3):
            rest += dcw[:, tap, :].T @ vTf[:, tap:tap + n_tok]
        y = (attn * rp1 + rest) * o_cm
        outs.append(np.stack([wp[:, 0:128].T @ y, wp[:, 128:256].T @ y]))

    n_tok = outs[0].shape[2]
    full = np.zeros((B, n_tok, 256), np.float32)
    for b in range(B):
        s = outs[2 * b] + outs[2 * b + 1]
        full[b] = s.reshape(256, n_tok).T
    return full


if __name__ == "__main__" and os.environ.get("KERNEL_SELFTEST"):
    sys.path.insert(0, os.path.dirname(os.path.abspath(__file__)))
    os.environ.setdefault("JAX_PLATFORMS", "cpu")
    import reference
    inputs = {k: np.asarray(v) for k, v in reference.setup_inputs().items()}
    expected = np.asarray(reference.reference(**inputs))
    per_core, use_bias = _host_prep(**inputs)
    got = _numpy_pipeline(per_core)
    err = np.abs(got - expected)
    rel = np.linalg.norm(got - expected) / np.linalg.norm(expected)
    print("selftest rel err:", rel, "max abs:", err.max())

if __name__ == "__main__" and os.environ.get("KERNEL_BUILD"):
    nc = _build_nc(False)
    import tempfile
    from concourse.bass_utils import compile_bass_kernel
    print("NEFF:", compile_bass_kernel(nc, tempfile.mkdtemp()))
